# revision 21
# baseline (speedup 1.0000x reference)
"""AGF layer (softmax-adjacency graph filter) on 8 TRN2 NeuronCores.

Math per (batch b, head h):
  q = x Wq ; k = x Wk ; v = x Wv                     (per-head 32-dim slices)
  A = softmax(q k^T / sqrt(32))                      [N, N]
  out_h = sum_k c[h,k] A^k v                         (K_ORDER = 3)
  out = concat_h(out_h) Wo + bo                      (biases are zeros by spec)

Sharding: core c handles batch b = c//2 and heads 4*(c%2)..4*(c%2)+3.

Design: per-HEAD software pipeline keyed off the ACT (scalar) engine, which
owns the exp() of the N x N score matrix (~37us/head; the hard serial floor).
While head j's scores stream PE->psum->ACT->E(fp8, SBUF), head j-1's filter
chain (s1 -> s2 -> s3 -> proj) runs on the PE in the gaps, and head j+1's
q/k projections slot in behind it.  Tail = only head 3's filter.

Normalization: softmax denominators ride a fused ones-column in the s1
stationary (consistent with the stored fp8 E).  They are extracted from the
s1 psum rows, reciprocal'd full-lane in f32, flattened via an SBUF->SBUF DMA
and broadcast across partitions with tiny K=1 ones-matmuls into f32 `rb`
tiles.  Each filter step's psum evacuation is then a single tensor_tensor
multiply by rb (normalize fused with the copy); transposed stationaries for
the next step come from XBAR transpose-DMAs.  The normalized `u` tiles are
already in [dims, tokens] layout, so the output projection needs no back-
transposes.  No PE-transposes anywhere.
"""
import sys
import numpy as np
import ml_dtypes

sys.path.insert(0, "/opt/trn_rl_repo")

import concourse.bass as bass
import concourse.mybir as mybir
from concourse import bacc, tile
from concourse.bass_utils import run_bass_kernel_spmd

BF16 = mybir.dt.bfloat16
F32 = mybir.dt.float32
FP8 = mybir.dt.float8e4

B, N, D, H, HD, KORD = 4, 2048, 256, 8, 32, 3
NB = N // 128          # 16 blocks of 128
SCALE = 1.0 / np.sqrt(HD)

Exp = mybir.ActivationFunctionType.Exp
MULT = mybir.AluOpType.mult

# bisection knobs
INTERLEAVE = True
SKIP_FILTER = False
SKIP_S23 = False
SKIP_PROJ = False


def build_graph():
    nc = bacc.Bacc("TRN2", target_bir_lowering=False, debug=False, num_devices=8)

    xT = nc.dram_tensor("xT", [2, 128, N], BF16, kind="ExternalInput")
    WQ = nc.dram_tensor("WQ", [4, 128, 2, 128], BF16, kind="ExternalInput")
    WK = nc.dram_tensor("WK", [4, 128, 2, 128], BF16, kind="ExternalInput")
    WV = nc.dram_tensor("WV", [128, 2, 128], BF16, kind="ExternalInput")
    WOK = nc.dram_tensor("WOK", [4, 128, 4, 256], BF16, kind="ExternalInput")
    out_d = nc.dram_tensor("out", [4, 2, 128, N], BF16, kind="ExternalOutput")

    vbounce = nc.dram_tensor("vbounce", [128, N], BF16)

    from contextlib import ExitStack
    with tile.TileContext(nc) as tc, ExitStack() as ctx:
        wp = ctx.enter_context(tc.tile_pool(name="wp", bufs=1))
        xp = ctx.enter_context(tc.tile_pool(name="xp", bufs=1))
        qkp = ctx.enter_context(tc.tile_pool(name="qkp", bufs=2))
        ep = ctx.enter_context(tc.tile_pool(name="ep", bufs=2))
        vp = ctx.enter_context(tc.tile_pool(name="vp", bufs=1))
        up = ctx.enter_context(tc.tile_pool(name="up", bufs=2))
        tnp = ctx.enter_context(tc.tile_pool(name="tnp", bufs=2))
        rp = ctx.enter_context(tc.tile_pool(name="rp", bufs=2))
        ocp = ctx.enter_context(tc.tile_pool(name="ocp", bufs=3))
        # PSUM budget (8 banks): scores 2x[128,1024]=4, s1 2x[128,512]=2,
        # everything else (qkproj/rb/s2/s3/proj) shares one 2x[128,512] ring.
        sps = ctx.enter_context(tc.tile_pool(name="sps", bufs=2, space="PSUM"))
        s1ps = ctx.enter_context(tc.tile_pool(name="s1ps", bufs=2, space="PSUM"))
        fp = ctx.enter_context(tc.tile_pool(name="fp", bufs=2, space="PSUM"))

        # ---------------- setup: inputs to SBUF
        xk = []
        for i in range(2):
            t = xp.tile([128, N], BF16, tag="xk", name=f"xk{i}", bufs=2)
            nc.sync.dma_start(out=t, in_=xT[i])
            xk.append(t)

        wq_t, wk_t = [], []
        for j in range(4):
            tq = wp.tile([128, 2, 128], BF16, tag=f"wq{j}", name=f"wq{j}")
            nc.sync.dma_start(out=tq, in_=WQ[j])
            wq_t.append(tq)
            tk = wp.tile([128, 2, 128], BF16, tag=f"wk{j}", name=f"wk{j}")
            nc.sync.dma_start(out=tk, in_=WK[j])
            wk_t.append(tk)
        wv_t = wp.tile([128, 2, 128], BF16, tag="wv")
        nc.sync.dma_start(out=wv_t, in_=WV[:, :, :])
        wok_t = []
        for j in range(4):
            t = wp.tile([128, 4, 256], BF16, tag=f"wok{j}", name=f"wok{j}")
            nc.sync.dma_start(out=t, in_=WOK[j])
            wok_t.append(t)

        ones32f = wp.tile([1, 32], F32, tag="ones32f")
        nc.vector.memset(ones32f, 1.0)

        # ---------------- per-head state
        qT, kT, Et = {}, {}, {}
        u1t = {}            # (j, sp) -> [128, 512] bf16 (t1, band layout)
        u23 = {}            # (j, k) -> [128, 512] bf16 (t2/t3, 4-band layout)
        u0t = {}            # j -> v in 4-band layout
        rbs = {}            # j -> (rb2 list, rb4)
        tn1s = {}           # j -> [tn1a, tn1b]
        vT = vp.tile([128, N], BF16, tag="vT")
        v1 = vp.tile([128, NB, 4, 33], BF16, tag="v1")

        def gen_qkproj(j):
            """Replicated q/k projections for head j -> qT/kT [128, N]."""
            for (wt, tag) in ((wq_t[j], "qT"), (wk_t[j], "kT")):
                dst = qkp.tile([128, N], BF16, tag=tag, name=f"{tag}{j}")
                for u4 in range(4):
                    s = slice(u4 * 512, (u4 + 1) * 512)
                    ps = fp.tile([128, 512], F32, tag="fp", name=f"qk{j}{tag}{u4}")
                    nc.tensor.matmul(ps, wt[:, 0, :], xk[0][:, s],
                                     start=True, stop=False)
                    nc.tensor.matmul(ps, wt[:, 1, :], xk[1][:, s],
                                     start=False, stop=True)
                    nc.vector.tensor_copy(out=dst[:, s], in_=ps)
                    yield
                if tag == "qT":
                    qT[j] = dst
                else:
                    kT[j] = dst

        def gen_vchain():
            """vT (4-head packed), vnat via DMA bounce transpose, v1 + ones."""
            for u4 in range(4):
                s = slice(u4 * 512, (u4 + 1) * 512)
                ps = fp.tile([128, 512], F32, tag="fp", name=f"v{u4}")
                nc.tensor.matmul(ps, wv_t[:, 0, :], xk[0][:, s],
                                 start=True, stop=False)
                nc.tensor.matmul(ps, wv_t[:, 1, :], xk[1][:, s],
                                 start=False, stop=True)
                nc.vector.tensor_copy(out=vT[:, s], in_=ps)
                yield
            nc.sync.dma_start(out=vbounce[:, :], in_=vT)
            vnat = vp.tile([128, NB, 128], BF16, tag="vnat")
            nc.sync.dma_start_transpose(out=vnat, in_=vbounce[:, :])
            yield
            for j in range(4):
                nc.vector.tensor_copy(out=v1[:, :, j, 0:32],
                                      in_=vnat[:, :, 32 * j:32 * j + 32])
                nc.vector.memset(v1[:, :, j, 32:33], 1.0)
                yield
            # u0[j]: v in the 4-band chunk layout (proj k=0 moving operand)
            for j in range(4):
                t = up.tile([128, 512], BF16, tag=f"u0_{j}", name=f"u0_{j}",
                            bufs=1)
                for c in range(4):
                    nc.sync.dma_start(
                        out=t[32 * c:32 * c + 32, :],
                        in_=vT[32 * j:32 * j + 32, c * 512:(c + 1) * 512])
                u0t[j] = t
                yield

        s1banks = {}

        def gen_scores(j):
            """Scores + exp for head j (ACT-paced), with head j's s1
            accumulation trailing one mb behind the exp stream.

            For head 0, s1 starts only at mb=8 (when the v-chain, emitted
            as background during these scores, is guaranteed done) and
            catches up two mb per step."""
            E = ep.tile([128, NB, N], FP8, tag="E", name=f"E{j}")
            Et[j] = E
            tq, tk = qT[j], kT[j]
            banks = [s1ps.tile([128, 512], F32, tag="fb", name=f"s1b{j}{sp}")
                     for sp in range(2)]
            s1banks[j] = banks

            def s1_mbs(mb):
                if j > 0:
                    return [(mb, mb == 0, mb == NB - 1)]
                if mb < 8:
                    return []
                return [(mb, mb == 8, False), (mb - 8, False, mb == NB - 1)]

            for mb in range(NB):
                for u in range(2):
                    ps = sps.tile([128, 1024], F32, tag="S", name=f"sc{j}{mb}{u}")
                    for half in range(2):
                        r = 2 * u + half
                        nc.tensor.matmul(
                            ps[:, half * 512:(half + 1) * 512],
                            tk[32 * r:32 * r + 32, mb * 128:(mb + 1) * 128],
                            tq[32 * r:32 * r + 32, r * 512:(r + 1) * 512],
                            start=True, stop=True, tile_position=(32 * r, 0))
                    nc.scalar.activation(out=E[:, mb, u * 1024:(u + 1) * 1024],
                                         in_=ps, func=Exp)
                    yield
                for (m1, st, sp_) in s1_mbs(mb):
                    for sp in range(2):
                        for hb in range(2):
                            c = 2 * sp + hb
                            nc.tensor.matmul(
                                banks[sp][64 * hb:64 * hb + 33, :],
                                v1[:, m1, j, :],
                                E[:, m1, c * 512:(c + 1) * 512],
                                start=st, stop=sp_, tile_position=(0, 64 * hb),
                                skip_group_check=True)

        def gen_rb_evac(j):
            """Small block emitted between score phases: denominators ->
            rb tiles (broadcast then full-lane reciprocal), s1 evacuation
            (normalize fused) and tn1 transposes.  Executes under the next
            head's score stream; frees the s1 psum banks."""
            banks = s1banks[j]
            dsb = rp.tile([1, 4, 512], F32, tag="dsb", name=f"dsb{j}")
            for c in range(4):
                sp, hb = c // 2, c % 2
                nc.vector.tensor_copy(
                    out=dsb[:, c, :],
                    in_=banks[sp][32 + 64 * hb:33 + 64 * hb, :])
            rps4 = fp.tile([128, 512], F32, tag="fp", name=f"rbp4{j}")
            for c in range(4):
                nc.tensor.matmul(rps4[32 * c:32 * c + 32, :], ones32f[:, :],
                                 dsb[:, c, :], start=True, stop=True,
                                 tile_position=(0, 32 * c))
            rb4 = rp.tile([128, 512], F32, tag="rb4", name=f"rb4{j}")
            nc.vector.reciprocal(out=rb4, in_=rps4)
            # rb2[sp] (bands 0/64 <- chunks 2sp/2sp+1) via partition-shift
            # DMA copies from rb4 (DVE cannot shift partitions; DMA can)
            rb2 = []
            for sp in range(2):
                rbt = rp.tile([128, 512], F32, tag=f"rb2_{sp}", name=f"rb2{j}{sp}")
                for hb in range(2):
                    c = 2 * sp + hb
                    nc.sync.dma_start(out=rbt[64 * hb:64 * hb + 32, :],
                                      in_=rb4[32 * c:32 * c + 32, :])
                rb2.append(rbt)
            rbs[j] = (rb2, rb4)
            # evacuate s1 (normalize fused) + transpose for s2 stationary
            tn1 = []
            for sp in range(2):
                ut = up.tile([128, 512], BF16, tag=f"u1_{sp}", name=f"u1{j}{sp}")
                nc.vector.memset(ut[32:64, :], 0.0)
                for hb in range(2):
                    nc.vector.tensor_tensor(
                        out=ut[64 * hb:64 * hb + 32, :],
                        in0=banks[sp][64 * hb:64 * hb + 32, :],
                        in1=rb2[sp][64 * hb:64 * hb + 32, :], op=MULT)
                u1t[(j, sp)] = ut
                tn = tnp.tile([128, 4, 96], BF16, tag=f"tn1_{sp}",
                              name=f"tn1{j}{sp}")
                nc.sync.dma_start_transpose(out=tn, in_=ut[0:96, :])
                tn1.append(tn)
            tn1s[j] = tn1

        def gen_rest(j):
            """s2 -> s3 -> proj for head j (emitted as background during
            head j+1's scores)."""
            E = Et[j]
            rb2, rb4 = rbs[j]
            tn1 = tn1s[j]

            def tn1_slice(mb):
                c, blk = mb // 4, mb % 4
                return tn1[c // 2][:, blk, 64 * (c % 2):64 * (c % 2) + 32]

            def s_step(k, stat_slice):
                bank = fp.tile([128, 512], F32, tag="fp", name=f"s{k}b{j}")
                for mb in range(NB):
                    st, sp_ = (mb == 0), (mb == NB - 1)
                    for c4 in range(4):
                        nc.tensor.matmul(
                            bank[32 * c4:32 * c4 + 32, :],
                            stat_slice(mb),
                            E[:, mb, c4 * 512:(c4 + 1) * 512],
                            start=st, stop=sp_, tile_position=(0, 32 * c4),
                            skip_group_check=True)
                    if mb % 2:
                        yield
                ut = up.tile([128, 512], BF16, tag=f"u{k}", name=f"u{k}_{j}")
                nc.vector.tensor_tensor(out=ut, in0=bank, in1=rb4, op=MULT)
                u23[(j, k)] = ut
                yield

            if SKIP_S23:
                return
            yield from s_step(2, tn1_slice)
            tn2 = tnp.tile([128, 4, 128], BF16, tag="tn2", name=f"tn2{j}")
            nc.sync.dma_start_transpose(out=tn2, in_=u23[(j, 2)])
            yield

            def tn2_slice(mb):
                return tn2[:, mb % 4, 32 * (mb // 4):32 * (mb // 4) + 32]

            yield from s_step(3, tn2_slice)

            if SKIP_PROJ:
                return
            # relayout t1 into the 4-band chunk layout (HW requires a fixed
            # tile_position within one accumulation chain)
            u1q = up.tile([128, 512], BF16, tag="u1q", name=f"u1q{j}")
            for c in range(4):
                b1 = 64 * (c % 2)
                nc.sync.dma_start(out=u1q[32 * c:32 * c + 32, :],
                                  in_=u1t[(j, c // 2)][b1:b1 + 32, :])
            yield
            # ---- output projection (+ DMA out per chunk)
            wok = wok_t[j]
            movs = (u0t[j], u1q, u23[(j, 2)], u23[(j, 3)])
            for mc in range(2):
                for c in range(4):
                    pp = fp.tile([128, 512], F32, tag="fp", name=f"pj{j}{mc}{c}")
                    cs = slice(mc * 128, (mc + 1) * 128)
                    b2 = 32 * c
                    for k in range(4):
                        nc.tensor.matmul(pp, wok[b2:b2 + 32, k, cs],
                                         movs[k][b2:b2 + 32, :],
                                         start=(k == 0), stop=(k == 3),
                                         tile_position=(b2, 0))
                    oc = ocp.tile([128, 512], BF16, tag="oc", name=f"oc{j}{mc}{c}")
                    nc.vector.tensor_copy(out=oc, in_=pp)
                    nc.sync.dma_start(out=out_d[j, mc, :, c * 512:(c + 1) * 512],
                                      in_=oc)
                    yield

        # ---------------- emission schedule
        def drain(g):
            for _ in g:
                pass

        def chain(*gens):
            for g in gens:
                yield from g

        def interleave(main, bg, ratio=1):
            while True:
                try:
                    next(main)
                except StopIteration:
                    break
                if bg is not None:
                    for _ in range(ratio):
                        try:
                            next(bg)
                        except StopIteration:
                            bg = None
                            break
            if bg is not None:
                drain(bg)

        def rest(j):
            return gen_rest(j) if not SKIP_FILTER else iter(())

        if INTERLEAVE:
            drain(gen_qkproj(0))
            interleave(gen_scores(0), chain(gen_vchain(), gen_qkproj(1)),
                       ratio=1)
            gen_rb_evac(0)
            interleave(gen_scores(1), chain(gen_qkproj(2), rest(0)), ratio=2)
            gen_rb_evac(1)
            interleave(gen_scores(2), chain(gen_qkproj(3), rest(1)), ratio=2)
            gen_rb_evac(2)
            interleave(gen_scores(3), rest(2), ratio=2)
            gen_rb_evac(3)
            drain(rest(3))
        else:
            drain(gen_qkproj(0))
            drain(gen_vchain())
            for j in range(4):
                drain(gen_scores(j))
                gen_rb_evac(j)
                if j < 3:
                    drain(gen_qkproj(j + 1))
                drain(rest(j))

    nc.compile()
    return nc


_graph_cache = None


def _get_graph():
    global _graph_cache
    if _graph_cache is None:
        _graph_cache = build_graph()
    return _graph_cache


# ---------------------------------------------------------------- host ----
def _prep_core_inputs(c, x, Wq, bq, Wk, bk, Wv, bv, Wo, coeffs):
    bf = ml_dtypes.bfloat16
    b, hh = c // 2, c % 2
    heads = [4 * hh + j for j in range(4)]

    xTb = np.ascontiguousarray(x[b].T.astype(np.float32)).reshape(2, 128, N)

    def rep4_rearr(Wcols):  # [256, 32] -> tiled x4 -> [128, 2, 128]
        wrep = np.tile(Wcols, (1, 4))                      # [256, 128]
        return np.ascontiguousarray(
            wrep.reshape(2, 128, 128).transpose(1, 0, 2))  # [128, 2, 128]

    WQc = np.stack([rep4_rearr(Wq[:, 32 * h:32 * h + 32] * SCALE)
                    for h in heads]).astype(bf)            # [4, 128, 2, 128]
    WKc = np.stack([rep4_rearr(Wk[:, 32 * h:32 * h + 32])
                    for h in heads]).astype(bf)

    wv_cols = np.concatenate([Wv[:, 32 * h:32 * h + 32] for h in heads], 1)
    WVc = np.ascontiguousarray(
        wv_cols.reshape(2, 128, 128).transpose(1, 0, 2)).astype(bf)

    WOKc = np.zeros((4, 128, 4, 256), np.float32)
    for j, h in enumerate(heads):
        rows = Wo[32 * h:32 * h + 32, :]                   # [32, 256]
        for k in range(4):
            WOKc[j, :, k, :] = np.tile(coeffs[h, k] * rows, (4, 1))

    return {"xT": xTb.astype(bf), "WQ": WQc, "WK": WKc,
            "WV": WVc, "WOK": WOKc.astype(bf)}


def kernel(**inputs):
    x = np.asarray(inputs["x"], np.float32)
    Wq = np.asarray(inputs["Wq"], np.float32)
    bq = np.asarray(inputs["bq"], np.float32)
    Wk = np.asarray(inputs["Wk"], np.float32)
    bk = np.asarray(inputs["bk"], np.float32)
    Wv = np.asarray(inputs["Wv"], np.float32)
    bv = np.asarray(inputs["bv"], np.float32)
    Wo = np.asarray(inputs["Wo"], np.float32)
    bo = np.asarray(inputs["bo"], np.float32)
    coeffs = np.asarray(inputs["coeffs"], np.float32)

    nc = _get_graph()
    in_maps = [_prep_core_inputs(c, x, Wq, bq, Wk, bk, Wv, bv, Wo, coeffs)
               for c in range(8)]
    res = run_bass_kernel_spmd(nc, in_maps, core_ids=list(range(8))).results

    out = np.zeros((B, N, D), np.float32)
    for c in range(8):
        o = np.asarray(res[c]["out"], np.float32)     # [4, 2, 128, N]
        out[c // 2] += o.sum(axis=0).reshape(256, N).T
    out += bo[None, None, :]
    return out


# revision 23
# speedup vs baseline: 1.0467x; 1.0467x over previous
"""AGF layer (softmax-adjacency graph filter) on 8 TRN2 NeuronCores.

Math per (batch b, head h):
  q = x Wq ; k = x Wk ; v = x Wv                     (per-head 32-dim slices)
  A = softmax(q k^T / sqrt(32))                      [N, N]
  out_h = sum_k c[h,k] A^k v                         (K_ORDER = 3)
  out = concat_h(out_h) Wo + bo                      (biases are zeros by spec)

Sharding: core c handles batch b = c//2 and heads 4*(c%2)..4*(c%2)+3.

Design: per-HEAD software pipeline keyed off the ACT (scalar) engine, which
owns the exp() of the N x N score matrix (~37us/head; the hard serial floor).
While head j's scores stream PE->psum->ACT->E(fp8, SBUF), head j-1's filter
chain (s1 -> s2 -> s3 -> proj) runs on the PE in the gaps, and head j+1's
q/k projections slot in behind it.  Tail = only head 3's filter.

Normalization: softmax denominators ride a fused ones-column in the s1
stationary (consistent with the stored fp8 E).  They are extracted from the
s1 psum rows, reciprocal'd full-lane in f32, flattened via an SBUF->SBUF DMA
and broadcast across partitions with tiny K=1 ones-matmuls into f32 `rb`
tiles.  Each filter step's psum evacuation is then a single tensor_tensor
multiply by rb (normalize fused with the copy); transposed stationaries for
the next step come from XBAR transpose-DMAs.  The normalized `u` tiles are
already in [dims, tokens] layout, so the output projection needs no back-
transposes.  No PE-transposes anywhere.
"""
import sys
import numpy as np
import ml_dtypes

sys.path.insert(0, "/opt/trn_rl_repo")

import concourse.bass as bass
import concourse.mybir as mybir
from concourse import bacc, tile
from concourse.bass_utils import run_bass_kernel_spmd

BF16 = mybir.dt.bfloat16
F32 = mybir.dt.float32
FP8 = mybir.dt.float8e4

B, N, D, H, HD, KORD = 4, 2048, 256, 8, 32, 3
NB = N // 128          # 16 blocks of 128
SCALE = 1.0 / np.sqrt(HD)

Exp = mybir.ActivationFunctionType.Exp
MULT = mybir.AluOpType.mult

# bisection knobs
WARMTH = 6          # dummy LDWEIGHTS per score quantum (HAM warmth keeper)
INTERLEAVE = True
SKIP_FILTER = False
SKIP_S23 = False
SKIP_PROJ = False


def build_graph():
    nc = bacc.Bacc("TRN2", target_bir_lowering=False, debug=False, num_devices=8)

    xT = nc.dram_tensor("xT", [2, 128, N], BF16, kind="ExternalInput")
    WQ = nc.dram_tensor("WQ", [4, 128, 2, 128], BF16, kind="ExternalInput")
    WK = nc.dram_tensor("WK", [4, 128, 2, 128], BF16, kind="ExternalInput")
    WV = nc.dram_tensor("WV", [128, 2, 128], BF16, kind="ExternalInput")
    WOK = nc.dram_tensor("WOK", [4, 128, 4, 256], BF16, kind="ExternalInput")
    out_d = nc.dram_tensor("out", [4, 2, 128, N], BF16, kind="ExternalOutput")

    vbounce = nc.dram_tensor("vbounce", [128, N], BF16)

    from contextlib import ExitStack
    with tile.TileContext(nc) as tc, ExitStack() as ctx:
        wp = ctx.enter_context(tc.tile_pool(name="wp", bufs=1))
        xp = ctx.enter_context(tc.tile_pool(name="xp", bufs=1))
        qkp = ctx.enter_context(tc.tile_pool(name="qkp", bufs=2))
        ep = ctx.enter_context(tc.tile_pool(name="ep", bufs=2))
        vp = ctx.enter_context(tc.tile_pool(name="vp", bufs=1))
        up = ctx.enter_context(tc.tile_pool(name="up", bufs=2))
        tnp = ctx.enter_context(tc.tile_pool(name="tnp", bufs=2))
        rp = ctx.enter_context(tc.tile_pool(name="rp", bufs=2))
        ocp = ctx.enter_context(tc.tile_pool(name="ocp", bufs=3))
        # PSUM budget (8 banks): scores 2x[128,1024]=4, s1 2x[128,512]=2,
        # everything else (qkproj/rb/s2/s3/proj) shares one 2x[128,512] ring.
        sps = ctx.enter_context(tc.tile_pool(name="sps", bufs=2, space="PSUM"))
        s1ps = ctx.enter_context(tc.tile_pool(name="s1ps", bufs=2, space="PSUM"))
        fp = ctx.enter_context(tc.tile_pool(name="fp", bufs=2, space="PSUM"))

        # ---------------- setup: inputs to SBUF
        xk = []
        for i in range(2):
            t = xp.tile([128, N], BF16, tag="xk", name=f"xk{i}", bufs=2)
            nc.sync.dma_start(out=t, in_=xT[i])
            xk.append(t)

        wq_t, wk_t = [], []
        for j in range(4):
            tq = wp.tile([128, 2, 128], BF16, tag=f"wq{j}", name=f"wq{j}")
            nc.sync.dma_start(out=tq, in_=WQ[j])
            wq_t.append(tq)
            tk = wp.tile([128, 2, 128], BF16, tag=f"wk{j}", name=f"wk{j}")
            nc.sync.dma_start(out=tk, in_=WK[j])
            wk_t.append(tk)
        wv_t = wp.tile([128, 2, 128], BF16, tag="wv")
        nc.sync.dma_start(out=wv_t, in_=WV[:, :, :])
        wok_t = []
        for j in range(4):
            t = wp.tile([128, 4, 256], BF16, tag=f"wok{j}", name=f"wok{j}")
            nc.sync.dma_start(out=t, in_=WOK[j])
            wok_t.append(t)

        ones32f = wp.tile([1, 32], F32, tag="ones32f")
        nc.vector.memset(ones32f, 1.0)

        # ---------------- per-head state
        qT, kT, Et = {}, {}, {}
        u1t = {}            # (j, sp) -> [128, 512] bf16 (t1, band layout)
        u23 = {}            # (j, k) -> [128, 512] bf16 (t2/t3, 4-band layout)
        u0t = {}            # j -> v in 4-band layout
        rbs = {}            # j -> (rb2 list, rb4)
        tn1s = {}           # j -> [tn1a, tn1b]
        vT = vp.tile([128, N], BF16, tag="vT")
        v1 = vp.tile([128, NB, 4, 33], BF16, tag="v1")

        def gen_qkproj(j):
            """Replicated q/k projections for head j -> qT/kT [128, N]."""
            for (wt, tag) in ((wq_t[j], "qT"), (wk_t[j], "kT")):
                dst = qkp.tile([128, N], BF16, tag=tag, name=f"{tag}{j}")
                for u4 in range(4):
                    s = slice(u4 * 512, (u4 + 1) * 512)
                    ps = fp.tile([128, 512], F32, tag="fp", name=f"qk{j}{tag}{u4}")
                    nc.tensor.matmul(ps, wt[:, 0, :], xk[0][:, s],
                                     start=True, stop=False)
                    nc.tensor.matmul(ps, wt[:, 1, :], xk[1][:, s],
                                     start=False, stop=True)
                    nc.vector.tensor_copy(out=dst[:, s], in_=ps)
                    yield
                if tag == "qT":
                    qT[j] = dst
                else:
                    kT[j] = dst

        def gen_vchain():
            """vT (4-head packed), vnat via DMA bounce transpose, v1 + ones."""
            for u4 in range(4):
                s = slice(u4 * 512, (u4 + 1) * 512)
                ps = fp.tile([128, 512], F32, tag="fp", name=f"v{u4}")
                nc.tensor.matmul(ps, wv_t[:, 0, :], xk[0][:, s],
                                 start=True, stop=False)
                nc.tensor.matmul(ps, wv_t[:, 1, :], xk[1][:, s],
                                 start=False, stop=True)
                nc.vector.tensor_copy(out=vT[:, s], in_=ps)
                yield
            nc.sync.dma_start(out=vbounce[:, :], in_=vT)
            vnat = vp.tile([128, NB, 128], BF16, tag="vnat")
            nc.sync.dma_start_transpose(out=vnat, in_=vbounce[:, :])
            yield
            for j in range(4):
                nc.vector.tensor_copy(out=v1[:, :, j, 0:32],
                                      in_=vnat[:, :, 32 * j:32 * j + 32])
                nc.vector.memset(v1[:, :, j, 32:33], 1.0)
                yield
            # u0[j]: v in the 4-band chunk layout (proj k=0 moving operand)
            for j in range(4):
                t = up.tile([128, 512], BF16, tag=f"u0_{j}", name=f"u0_{j}",
                            bufs=1)
                for c in range(4):
                    nc.sync.dma_start(
                        out=t[32 * c:32 * c + 32, :],
                        in_=vT[32 * j:32 * j + 32, c * 512:(c + 1) * 512])
                u0t[j] = t
                yield

        s1banks = {}

        def gen_scores(j):
            """Scores + exp for head j (ACT-paced), with head j's s1
            accumulation trailing one mb behind the exp stream.

            For head 0, s1 starts only at mb=8 (when the v-chain, emitted
            as background during these scores, is guaranteed done) and
            catches up two mb per step."""
            E = ep.tile([128, NB, N], FP8, tag="E", name=f"E{j}")
            Et[j] = E
            tq, tk = qT[j], kT[j]
            banks = [s1ps.tile([128, 512], F32, tag="fb", name=f"s1b{j}{sp}")
                     for sp in range(2)]
            s1banks[j] = banks

            def s1_mbs(mb):
                if j > 0:
                    return [(mb, mb == 0, mb == NB - 1)]
                return []          # head 0: s1 runs in scores(1)'s background

            for mb in range(NB):
                for u in range(2):
                    ps = sps.tile([128, 1024], F32, tag="S", name=f"sc{j}{mb}{u}")
                    for half in range(2):
                        r = 2 * u + half
                        nc.tensor.matmul(
                            ps[:, half * 512:(half + 1) * 512],
                            tk[32 * r:32 * r + 32, mb * 128:(mb + 1) * 128],
                            tq[32 * r:32 * r + 32, r * 512:(r + 1) * 512],
                            start=True, stop=True, tile_position=(32 * r, 0))
                    nc.scalar.activation(out=E[:, mb, u * 1024:(u + 1) * 1024],
                                         in_=ps, func=Exp)
                    for _w in range(WARMTH):
                        nc.tensor.ldweights(weights=wq_t[0][:, 0, :])
                    yield
                for (m1, st, sp_) in s1_mbs(mb):
                    for sp in range(2):
                        for hb in range(2):
                            c = 2 * sp + hb
                            nc.tensor.matmul(
                                banks[sp][64 * hb:64 * hb + 33, :],
                                v1[:, m1, j, :],
                                E[:, m1, c * 512:(c + 1) * 512],
                                start=st, stop=sp_, tile_position=(0, 64 * hb),
                                skip_group_check=True)

        def gen_s1_late(j):
            """s1 accumulation for head j from fully-materialized E
            (used for head 0, whose v1 isn't ready during its scores)."""
            E = Et[j]
            banks = s1banks[j]
            for mb in range(NB):
                st, sp_ = (mb == 0), (mb == NB - 1)
                for sp in range(2):
                    for hb in range(2):
                        c = 2 * sp + hb
                        nc.tensor.matmul(
                            banks[sp][64 * hb:64 * hb + 33, :],
                            v1[:, mb, j, :],
                            E[:, mb, c * 512:(c + 1) * 512],
                            start=st, stop=sp_, tile_position=(0, 64 * hb),
                            skip_group_check=True)
                if mb % 2:
                    yield

        def gen_rb_evac(j):
            """Small block emitted between score phases: denominators ->
            rb tiles (broadcast then full-lane reciprocal), s1 evacuation
            (normalize fused) and tn1 transposes.  Executes under the next
            head's score stream; frees the s1 psum banks."""
            banks = s1banks[j]
            dsb = rp.tile([1, 4, 512], F32, tag="dsb", name=f"dsb{j}")
            for c in range(4):
                sp, hb = c // 2, c % 2
                nc.vector.tensor_copy(
                    out=dsb[:, c, :],
                    in_=banks[sp][32 + 64 * hb:33 + 64 * hb, :])
            rps4 = fp.tile([128, 512], F32, tag="fp", name=f"rbp4{j}")
            for c in range(4):
                nc.tensor.matmul(rps4[32 * c:32 * c + 32, :], ones32f[:, :],
                                 dsb[:, c, :], start=True, stop=True,
                                 tile_position=(0, 32 * c))
            rb4 = rp.tile([128, 512], F32, tag="rb4", name=f"rb4{j}")
            nc.vector.reciprocal(out=rb4, in_=rps4)
            # rb2[sp] (bands 0/64 <- chunks 2sp/2sp+1) via partition-shift
            # DMA copies from rb4 (DVE cannot shift partitions; DMA can)
            rb2 = []
            for sp in range(2):
                rbt = rp.tile([128, 512], F32, tag=f"rb2_{sp}", name=f"rb2{j}{sp}")
                for hb in range(2):
                    c = 2 * sp + hb
                    nc.sync.dma_start(out=rbt[64 * hb:64 * hb + 32, :],
                                      in_=rb4[32 * c:32 * c + 32, :])
                rb2.append(rbt)
            rbs[j] = (rb2, rb4)
            # evacuate s1 (normalize fused) + transpose for s2 stationary
            tn1 = []
            for sp in range(2):
                ut = up.tile([128, 512], BF16, tag=f"u1_{sp}", name=f"u1{j}{sp}")
                nc.vector.memset(ut[32:64, :], 0.0)
                for hb in range(2):
                    nc.vector.tensor_tensor(
                        out=ut[64 * hb:64 * hb + 32, :],
                        in0=banks[sp][64 * hb:64 * hb + 32, :],
                        in1=rb2[sp][64 * hb:64 * hb + 32, :], op=MULT)
                u1t[(j, sp)] = ut
                tn = tnp.tile([128, 4, 96], BF16, tag=f"tn1_{sp}",
                              name=f"tn1{j}{sp}")
                nc.sync.dma_start_transpose(out=tn, in_=ut[0:96, :])
                tn1.append(tn)
            tn1s[j] = tn1

        def gen_rest(j):
            """s2 -> s3 -> proj for head j (emitted as background during
            head j+1's scores)."""
            E = Et[j]
            rb2, rb4 = rbs[j]
            tn1 = tn1s[j]

            def tn1_slice(mb):
                c, blk = mb // 4, mb % 4
                return tn1[c // 2][:, blk, 64 * (c % 2):64 * (c % 2) + 32]

            def s_step(k, stat_slice):
                bank = fp.tile([128, 512], F32, tag="fp", name=f"s{k}b{j}")
                for mb in range(NB):
                    st, sp_ = (mb == 0), (mb == NB - 1)
                    for c4 in range(4):
                        nc.tensor.matmul(
                            bank[32 * c4:32 * c4 + 32, :],
                            stat_slice(mb),
                            E[:, mb, c4 * 512:(c4 + 1) * 512],
                            start=st, stop=sp_, tile_position=(0, 32 * c4),
                            skip_group_check=True)
                    yield
                ut = up.tile([128, 512], BF16, tag=f"u{k}", name=f"u{k}_{j}")
                nc.vector.tensor_tensor(out=ut, in0=bank, in1=rb4, op=MULT)
                u23[(j, k)] = ut
                yield

            if SKIP_S23:
                return
            yield from s_step(2, tn1_slice)
            tn2 = tnp.tile([128, 4, 128], BF16, tag="tn2", name=f"tn2{j}")
            nc.sync.dma_start_transpose(out=tn2, in_=u23[(j, 2)])
            yield

            def tn2_slice(mb):
                return tn2[:, mb % 4, 32 * (mb // 4):32 * (mb // 4) + 32]

            yield from s_step(3, tn2_slice)

            if SKIP_PROJ:
                return
            # relayout t1 into the 4-band chunk layout (HW requires a fixed
            # tile_position within one accumulation chain)
            u1q = up.tile([128, 512], BF16, tag="u1q", name=f"u1q{j}")
            for c in range(4):
                b1 = 64 * (c % 2)
                nc.sync.dma_start(out=u1q[32 * c:32 * c + 32, :],
                                  in_=u1t[(j, c // 2)][b1:b1 + 32, :])
            yield
            # ---- output projection (+ DMA out per chunk)
            wok = wok_t[j]
            movs = (u0t[j], u1q, u23[(j, 2)], u23[(j, 3)])
            for mc in range(2):
                for c in range(4):
                    pp = fp.tile([128, 512], F32, tag="fp", name=f"pj{j}{mc}{c}")
                    cs = slice(mc * 128, (mc + 1) * 128)
                    b2 = 32 * c
                    for k in range(4):
                        nc.tensor.matmul(pp, wok[b2:b2 + 32, k, cs],
                                         movs[k][b2:b2 + 32, :],
                                         start=(k == 0), stop=(k == 3),
                                         tile_position=(b2, 0))
                    oc = ocp.tile([128, 512], BF16, tag="oc", name=f"oc{j}{mc}{c}")
                    nc.vector.tensor_copy(out=oc, in_=pp)
                    nc.sync.dma_start(out=out_d[j, mc, :, c * 512:(c + 1) * 512],
                                      in_=oc)
                    yield

        # ---------------- emission schedule
        def drain(g):
            for _ in g:
                pass

        def chain(*gens):
            for g in gens:
                yield from g

        def interleave(main, bg, ratio=1):
            while True:
                try:
                    next(main)
                except StopIteration:
                    break
                if bg is not None:
                    for _ in range(ratio):
                        try:
                            next(bg)
                        except StopIteration:
                            bg = None
                            break
            if bg is not None:
                drain(bg)

        def rest(j):
            return gen_rest(j) if not SKIP_FILTER else iter(())

        def rb_evac_gen(j):
            # lazy: emits rb_evac(j) when first pulled (inside a chain)
            gen_rb_evac(j)
            if False:
                yield

        if INTERLEAVE:
            drain(gen_qkproj(0))
            interleave(gen_scores(0), chain(gen_vchain(), gen_qkproj(1)),
                       ratio=1)
            interleave(gen_scores(1),
                       chain(gen_s1_late(0), rb_evac_gen(0), gen_qkproj(2),
                             rest(0)), ratio=2)
            gen_rb_evac(1)
            interleave(gen_scores(2), chain(gen_qkproj(3), rest(1)), ratio=2)
            gen_rb_evac(2)
            interleave(gen_scores(3), rest(2), ratio=2)
            gen_rb_evac(3)
            drain(rest(3))
        else:
            drain(gen_qkproj(0))
            drain(gen_vchain())
            for j in range(4):
                drain(gen_scores(j))
                if j == 0:
                    drain(gen_s1_late(0))
                gen_rb_evac(j)
                if j < 3:
                    drain(gen_qkproj(j + 1))
                drain(rest(j))

    nc.compile()
    return nc


_graph_cache = None


def _get_graph():
    global _graph_cache
    if _graph_cache is None:
        _graph_cache = build_graph()
    return _graph_cache


# ---------------------------------------------------------------- host ----
def _prep_core_inputs(c, x, Wq, bq, Wk, bk, Wv, bv, Wo, coeffs):
    bf = ml_dtypes.bfloat16
    b, hh = c // 2, c % 2
    heads = [4 * hh + j for j in range(4)]

    xTb = np.ascontiguousarray(x[b].T.astype(np.float32)).reshape(2, 128, N)

    def rep4_rearr(Wcols):  # [256, 32] -> tiled x4 -> [128, 2, 128]
        wrep = np.tile(Wcols, (1, 4))                      # [256, 128]
        return np.ascontiguousarray(
            wrep.reshape(2, 128, 128).transpose(1, 0, 2))  # [128, 2, 128]

    WQc = np.stack([rep4_rearr(Wq[:, 32 * h:32 * h + 32] * SCALE)
                    for h in heads]).astype(bf)            # [4, 128, 2, 128]
    WKc = np.stack([rep4_rearr(Wk[:, 32 * h:32 * h + 32])
                    for h in heads]).astype(bf)

    wv_cols = np.concatenate([Wv[:, 32 * h:32 * h + 32] for h in heads], 1)
    WVc = np.ascontiguousarray(
        wv_cols.reshape(2, 128, 128).transpose(1, 0, 2)).astype(bf)

    WOKc = np.zeros((4, 128, 4, 256), np.float32)
    for j, h in enumerate(heads):
        rows = Wo[32 * h:32 * h + 32, :]                   # [32, 256]
        for k in range(4):
            WOKc[j, :, k, :] = np.tile(coeffs[h, k] * rows, (4, 1))

    return {"xT": xTb.astype(bf), "WQ": WQc, "WK": WKc,
            "WV": WVc, "WOK": WOKc.astype(bf)}


def kernel(**inputs):
    x = np.asarray(inputs["x"], np.float32)
    Wq = np.asarray(inputs["Wq"], np.float32)
    bq = np.asarray(inputs["bq"], np.float32)
    Wk = np.asarray(inputs["Wk"], np.float32)
    bk = np.asarray(inputs["bk"], np.float32)
    Wv = np.asarray(inputs["Wv"], np.float32)
    bv = np.asarray(inputs["bv"], np.float32)
    Wo = np.asarray(inputs["Wo"], np.float32)
    bo = np.asarray(inputs["bo"], np.float32)
    coeffs = np.asarray(inputs["coeffs"], np.float32)

    nc = _get_graph()
    in_maps = [_prep_core_inputs(c, x, Wq, bq, Wk, bk, Wv, bv, Wo, coeffs)
               for c in range(8)]
    res = run_bass_kernel_spmd(nc, in_maps, core_ids=list(range(8))).results

    out = np.zeros((B, N, D), np.float32)
    for c in range(8):
        o = np.asarray(res[c]["out"], np.float32)     # [4, 2, 128, N]
        out[c // 2] += o.sum(axis=0).reshape(256, N).T
    out += bo[None, None, :]
    return out


# revision 24
# speedup vs baseline: 1.1995x; 1.1460x over previous
"""AGF layer (softmax-adjacency graph filter) on 8 TRN2 NeuronCores.

Math per (batch b, head h):
  q = x Wq ; k = x Wk ; v = x Wv                     (per-head 32-dim slices)
  A = softmax(q k^T / sqrt(32))                      [N, N]
  out_h = sum_k c[h,k] A^k v                         (K_ORDER = 3)
  out = concat_h(out_h) Wo + bo                      (biases are zeros by spec)

Sharding: core c handles batch b = c//2 and heads 4*(c%2)..4*(c%2)+3.

Design: per-HEAD software pipeline keyed off the ACT (scalar) engine, which
owns the exp() of the N x N score matrix (~37us/head; the hard serial floor).
While head j's scores stream PE->psum->ACT->E(fp8, SBUF), head j-1's filter
chain (s1 -> s2 -> s3 -> proj) runs on the PE in the gaps, and head j+1's
q/k projections slot in behind it.  Tail = only head 3's filter.

Normalization: softmax denominators ride a fused ones-column in the s1
stationary (consistent with the stored fp8 E).  They are extracted from the
s1 psum rows, reciprocal'd full-lane in f32, flattened via an SBUF->SBUF DMA
and broadcast across partitions with tiny K=1 ones-matmuls into f32 `rb`
tiles.  Each filter step's psum evacuation is then a single tensor_tensor
multiply by rb (normalize fused with the copy); transposed stationaries for
the next step come from XBAR transpose-DMAs.  The normalized `u` tiles are
already in [dims, tokens] layout, so the output projection needs no back-
transposes.  No PE-transposes anywhere.
"""
import sys
import numpy as np
import ml_dtypes

sys.path.insert(0, "/opt/trn_rl_repo")

import concourse.bass as bass
import concourse.mybir as mybir
from concourse import bacc, tile
from concourse.bass_utils import run_bass_kernel_spmd

BF16 = mybir.dt.bfloat16
F32 = mybir.dt.float32
FP8 = mybir.dt.float8e4

B, N, D, H, HD, KORD = 4, 2048, 256, 8, 32, 3
NB = N // 128          # 16 blocks of 128
SCALE = 1.0 / np.sqrt(HD)

Exp = mybir.ActivationFunctionType.Exp
MULT = mybir.AluOpType.mult

# bisection knobs
INTERLEAVE = True
SKIP_FILTER = False
SKIP_S23 = False
SKIP_PROJ = False


def build_graph():
    nc = bacc.Bacc("TRN2", target_bir_lowering=False, debug=False, num_devices=8)

    xT = nc.dram_tensor("xT", [2, 128, N], BF16, kind="ExternalInput")
    WQ = nc.dram_tensor("WQ", [4, 128, 2, 128], BF16, kind="ExternalInput")
    WK = nc.dram_tensor("WK", [4, 128, 2, 128], BF16, kind="ExternalInput")
    WV = nc.dram_tensor("WV", [128, 2, 128], BF16, kind="ExternalInput")
    WOK = nc.dram_tensor("WOK", [4, 128, 4, 256], BF16, kind="ExternalInput")
    out_d = nc.dram_tensor("out", [4, 2, 128, N], BF16, kind="ExternalOutput")

    vbounce = nc.dram_tensor("vbounce", [128, N], BF16)

    from contextlib import ExitStack
    with tile.TileContext(nc) as tc, ExitStack() as ctx:
        wp = ctx.enter_context(tc.tile_pool(name="wp", bufs=1))
        xp = ctx.enter_context(tc.tile_pool(name="xp", bufs=1))
        qkp = ctx.enter_context(tc.tile_pool(name="qkp", bufs=2))
        ep = ctx.enter_context(tc.tile_pool(name="ep", bufs=2))
        vp = ctx.enter_context(tc.tile_pool(name="vp", bufs=1))
        up = ctx.enter_context(tc.tile_pool(name="up", bufs=2))
        tnp = ctx.enter_context(tc.tile_pool(name="tnp", bufs=2))
        rp = ctx.enter_context(tc.tile_pool(name="rp", bufs=2))
        ocp = ctx.enter_context(tc.tile_pool(name="ocp", bufs=3))
        # PSUM budget (8 banks): scores 2x[128,1024]=4, s1 2x[128,512]=2,
        # everything else (qkproj/rb/s2/s3/proj) shares one 2x[128,512] ring.
        sps = ctx.enter_context(tc.tile_pool(name="sps", bufs=2, space="PSUM"))
        s1ps = ctx.enter_context(tc.tile_pool(name="s1ps", bufs=2, space="PSUM"))
        fp = ctx.enter_context(tc.tile_pool(name="fp", bufs=2, space="PSUM"))

        # ---------------- setup: inputs to SBUF
        xk = []
        for i in range(2):
            t = xp.tile([128, N], BF16, tag="xk", name=f"xk{i}", bufs=2)
            nc.sync.dma_start(out=t, in_=xT[i])
            xk.append(t)

        wq_t, wk_t = [], []
        for j in range(4):
            tq = wp.tile([128, 2, 128], BF16, tag=f"wq{j}", name=f"wq{j}")
            nc.sync.dma_start(out=tq, in_=WQ[j])
            wq_t.append(tq)
            tk = wp.tile([128, 2, 128], BF16, tag=f"wk{j}", name=f"wk{j}")
            nc.sync.dma_start(out=tk, in_=WK[j])
            wk_t.append(tk)
        wv_t = wp.tile([128, 2, 128], BF16, tag="wv")
        nc.sync.dma_start(out=wv_t, in_=WV[:, :, :])
        wok_t = []
        for j in range(4):
            t = wp.tile([128, 4, 256], BF16, tag=f"wok{j}", name=f"wok{j}")
            nc.sync.dma_start(out=t, in_=WOK[j])
            wok_t.append(t)

        ones32f = wp.tile([1, 32], F32, tag="ones32f")
        nc.vector.memset(ones32f, 1.0)

        # ---------------- per-head state
        qT, kT, Et = {}, {}, {}
        u1t = {}            # (j, sp) -> [128, 512] bf16 (t1, band layout)
        u23 = {}            # (j, k) -> [128, 512] bf16 (t2/t3, 4-band layout)
        u0t = {}            # j -> v in 4-band layout
        rbs = {}            # j -> (rb2 list, rb4)
        tn1s = {}           # j -> [tn1a, tn1b]
        vT = vp.tile([128, N], BF16, tag="vT")
        v1 = vp.tile([128, NB, 4, 33], BF16, tag="v1")

        def gen_qkproj(j):
            """Replicated q/k projections for head j -> qT/kT [128, N]."""
            for (wt, tag) in ((wq_t[j], "qT"), (wk_t[j], "kT")):
                dst = qkp.tile([128, N], BF16, tag=tag, name=f"{tag}{j}")
                for u4 in range(4):
                    s = slice(u4 * 512, (u4 + 1) * 512)
                    ps = fp.tile([128, 512], F32, tag="fp", name=f"qk{j}{tag}{u4}")
                    nc.tensor.matmul(ps, wt[:, 0, :], xk[0][:, s],
                                     start=True, stop=False)
                    nc.tensor.matmul(ps, wt[:, 1, :], xk[1][:, s],
                                     start=False, stop=True)
                    nc.vector.tensor_copy(out=dst[:, s], in_=ps)
                    yield
                if tag == "qT":
                    qT[j] = dst
                else:
                    kT[j] = dst

        def gen_vchain():
            """vT (4-head packed), vnat via DMA bounce transpose, v1 + ones."""
            for u4 in range(4):
                s = slice(u4 * 512, (u4 + 1) * 512)
                ps = fp.tile([128, 512], F32, tag="fp", name=f"v{u4}")
                nc.tensor.matmul(ps, wv_t[:, 0, :], xk[0][:, s],
                                 start=True, stop=False)
                nc.tensor.matmul(ps, wv_t[:, 1, :], xk[1][:, s],
                                 start=False, stop=True)
                nc.vector.tensor_copy(out=vT[:, s], in_=ps)
                yield
            nc.sync.dma_start(out=vbounce[:, :], in_=vT)
            vnat = vp.tile([128, NB, 128], BF16, tag="vnat")
            nc.sync.dma_start_transpose(out=vnat, in_=vbounce[:, :])
            yield
            for j in range(4):
                nc.vector.tensor_copy(out=v1[:, :, j, 0:32],
                                      in_=vnat[:, :, 32 * j:32 * j + 32])
                nc.vector.memset(v1[:, :, j, 32:33], 1.0)
                yield
            # u0[j]: v in the 4-band chunk layout (proj k=0 moving operand)
            for j in range(4):
                t = up.tile([128, 512], BF16, tag=f"u0_{j}", name=f"u0_{j}",
                            bufs=1)
                for c in range(4):
                    nc.sync.dma_start(
                        out=t[32 * c:32 * c + 32, :],
                        in_=vT[32 * j:32 * j + 32, c * 512:(c + 1) * 512])
                u0t[j] = t
                yield

        s1banks = {}

        def gen_scores(j):
            """Scores + exp for head j (ACT-paced), with head j's s1
            accumulation trailing one mb behind the exp stream.

            For head 0, s1 starts only at mb=8 (when the v-chain, emitted
            as background during these scores, is guaranteed done) and
            catches up two mb per step."""
            E = ep.tile([128, NB, N], FP8, tag="E", name=f"E{j}")
            Et[j] = E
            tq, tk = qT[j], kT[j]
            banks = [s1ps.tile([128, 512], F32, tag="fb", name=f"s1b{j}{sp}")
                     for sp in range(2)]
            s1banks[j] = banks

            def s1_mbs(mb):
                if j > 0:
                    return [(mb, mb == 0, mb == NB - 1)]
                return []          # head 0: s1 runs in scores(1)'s background

            for mb in range(NB):
                for u in range(2):
                    ps = sps.tile([128, 1024], F32, tag="S", name=f"sc{j}{mb}{u}")
                    for half in range(2):
                        r = 2 * u + half
                        nc.tensor.matmul(
                            ps[:, half * 512:(half + 1) * 512],
                            tk[32 * r:32 * r + 32, mb * 128:(mb + 1) * 128],
                            tq[32 * r:32 * r + 32, r * 512:(r + 1) * 512],
                            start=True, stop=True, tile_position=(32 * r, 0))
                    nc.scalar.activation(out=E[:, mb, u * 1024:(u + 1) * 1024],
                                         in_=ps, func=Exp)
                    yield
                for (m1, st, sp_) in s1_mbs(mb):
                    for sp in range(2):
                        for hb in range(2):
                            c = 2 * sp + hb
                            nc.tensor.matmul(
                                banks[sp][64 * hb:64 * hb + 33, :],
                                v1[:, m1, j, :],
                                E[:, m1, c * 512:(c + 1) * 512],
                                start=st, stop=sp_, tile_position=(0, 64 * hb),
                                skip_group_check=True)

        def gen_s1_late(j):
            """s1 accumulation for head j from fully-materialized E
            (used for head 0, whose v1 isn't ready during its scores)."""
            E = Et[j]
            banks = s1banks[j]
            for mb in range(NB):
                st, sp_ = (mb == 0), (mb == NB - 1)
                for sp in range(2):
                    for hb in range(2):
                        c = 2 * sp + hb
                        nc.tensor.matmul(
                            banks[sp][64 * hb:64 * hb + 33, :],
                            v1[:, mb, j, :],
                            E[:, mb, c * 512:(c + 1) * 512],
                            start=st, stop=sp_, tile_position=(0, 64 * hb),
                            skip_group_check=True)
                if mb % 2:
                    yield

        def gen_rb_evac(j):
            """Small block emitted between score phases: denominators ->
            rb tiles (broadcast then full-lane reciprocal), s1 evacuation
            (normalize fused) and tn1 transposes.  Executes under the next
            head's score stream; frees the s1 psum banks."""
            banks = s1banks[j]
            dsb = rp.tile([1, 4, 512], F32, tag="dsb", name=f"dsb{j}")
            for c in range(4):
                sp, hb = c // 2, c % 2
                nc.vector.tensor_copy(
                    out=dsb[:, c, :],
                    in_=banks[sp][32 + 64 * hb:33 + 64 * hb, :])
            rps4 = fp.tile([128, 512], F32, tag="fp", name=f"rbp4{j}")
            for c in range(4):
                nc.tensor.matmul(rps4[32 * c:32 * c + 32, :], ones32f[:, :],
                                 dsb[:, c, :], start=True, stop=True,
                                 tile_position=(0, 32 * c))
            rb4 = rp.tile([128, 512], F32, tag="rb4", name=f"rb4{j}")
            nc.vector.reciprocal(out=rb4, in_=rps4)
            # rb2[sp] (bands 0/64 <- chunks 2sp/2sp+1) via partition-shift
            # DMA copies from rb4 (DVE cannot shift partitions; DMA can)
            rb2 = []
            for sp in range(2):
                rbt = rp.tile([128, 512], F32, tag=f"rb2_{sp}", name=f"rb2{j}{sp}")
                for hb in range(2):
                    c = 2 * sp + hb
                    nc.sync.dma_start(out=rbt[64 * hb:64 * hb + 32, :],
                                      in_=rb4[32 * c:32 * c + 32, :])
                rb2.append(rbt)
            rbs[j] = (rb2, rb4)
            # evacuate s1 (normalize fused) + transpose for s2 stationary
            tn1 = []
            for sp in range(2):
                ut = up.tile([128, 512], BF16, tag=f"u1_{sp}", name=f"u1{j}{sp}")
                nc.vector.memset(ut[32:64, :], 0.0)
                for hb in range(2):
                    nc.vector.tensor_tensor(
                        out=ut[64 * hb:64 * hb + 32, :],
                        in0=banks[sp][64 * hb:64 * hb + 32, :],
                        in1=rb2[sp][64 * hb:64 * hb + 32, :], op=MULT)
                u1t[(j, sp)] = ut
                tn = tnp.tile([128, 4, 96], BF16, tag=f"tn1_{sp}",
                              name=f"tn1{j}{sp}")
                nc.sync.dma_start_transpose(out=tn, in_=ut[0:96, :])
                tn1.append(tn)
            tn1s[j] = tn1

        def gen_rest(j):
            """s2 -> s3 -> proj for head j (emitted as background during
            head j+1's scores)."""
            E = Et[j]
            rb2, rb4 = rbs[j]
            tn1 = tn1s[j]

            def tn1_slice(mb):
                c, blk = mb // 4, mb % 4
                return tn1[c // 2][:, blk, 64 * (c % 2):64 * (c % 2) + 32]

            def s_step(k, stat_slice):
                bank = fp.tile([128, 512], F32, tag="fp", name=f"s{k}b{j}")
                for mb in range(NB):
                    st, sp_ = (mb == 0), (mb == NB - 1)
                    for c4 in range(4):
                        nc.tensor.matmul(
                            bank[32 * c4:32 * c4 + 32, :],
                            stat_slice(mb),
                            E[:, mb, c4 * 512:(c4 + 1) * 512],
                            start=st, stop=sp_, tile_position=(0, 32 * c4),
                            skip_group_check=True)
                    yield
                ut = up.tile([128, 512], BF16, tag=f"u{k}", name=f"u{k}_{j}")
                nc.vector.tensor_tensor(out=ut, in0=bank, in1=rb4, op=MULT)
                u23[(j, k)] = ut
                yield

            if SKIP_S23:
                return
            yield from s_step(2, tn1_slice)
            tn2 = tnp.tile([128, 4, 128], BF16, tag="tn2", name=f"tn2{j}")
            nc.sync.dma_start_transpose(out=tn2, in_=u23[(j, 2)])
            yield

            def tn2_slice(mb):
                return tn2[:, mb % 4, 32 * (mb // 4):32 * (mb // 4) + 32]

            yield from s_step(3, tn2_slice)

            if SKIP_PROJ:
                return
            # relayout t1 into the 4-band chunk layout (HW requires a fixed
            # tile_position within one accumulation chain)
            u1q = up.tile([128, 512], BF16, tag="u1q", name=f"u1q{j}")
            for c in range(4):
                b1 = 64 * (c % 2)
                nc.sync.dma_start(out=u1q[32 * c:32 * c + 32, :],
                                  in_=u1t[(j, c // 2)][b1:b1 + 32, :])
            yield
            # ---- output projection (+ DMA out per chunk)
            wok = wok_t[j]
            movs = (u0t[j], u1q, u23[(j, 2)], u23[(j, 3)])
            for mc in range(2):
                cs = slice(mc * 128, (mc + 1) * 128)
                for c0 in (0, 2):
                    pps = [fp.tile([128, 512], F32, tag="fp",
                                   name=f"pj{j}{mc}{c0 + i}") for i in range(2)]
                    # interleave the two chains (distinct row bands) so the
                    # PE overlaps them
                    for k in range(4):
                        for i in range(2):
                            b2 = 32 * (c0 + i)
                            nc.tensor.matmul(pps[i], wok[b2:b2 + 32, k, cs],
                                             movs[k][b2:b2 + 32, :],
                                             start=(k == 0), stop=(k == 3),
                                             tile_position=(b2, 0))
                        yield
                    for i in range(2):
                        c = c0 + i
                        oc = ocp.tile([128, 512], BF16, tag="oc",
                                      name=f"oc{j}{mc}{c}")
                        nc.vector.tensor_copy(out=oc, in_=pps[i])
                        nc.sync.dma_start(
                            out=out_d[j, mc, :, c * 512:(c + 1) * 512], in_=oc)
                    yield

        # ---------------- emission schedule
        def drain(g):
            for _ in g:
                pass

        def chain(*gens):
            for g in gens:
                yield from g

        def interleave(main, bg, ratio=1):
            while True:
                try:
                    next(main)
                except StopIteration:
                    break
                if bg is not None:
                    for _ in range(ratio):
                        try:
                            next(bg)
                        except StopIteration:
                            bg = None
                            break
            if bg is not None:
                drain(bg)

        def rest(j):
            return gen_rest(j) if not SKIP_FILTER else iter(())

        def rb_evac_gen(j):
            # lazy: emits rb_evac(j) when first pulled (inside a chain)
            gen_rb_evac(j)
            if False:
                yield

        if INTERLEAVE:
            drain(gen_qkproj(0))
            interleave(gen_scores(0), chain(gen_vchain(), gen_qkproj(1)),
                       ratio=1)
            interleave(gen_scores(1),
                       chain(gen_s1_late(0), rb_evac_gen(0), gen_qkproj(2),
                             rest(0)), ratio=3)
            gen_rb_evac(1)
            interleave(gen_scores(2), chain(gen_qkproj(3), rest(1)), ratio=3)
            gen_rb_evac(2)
            interleave(gen_scores(3), rest(2), ratio=3)
            gen_rb_evac(3)
            drain(rest(3))
        else:
            drain(gen_qkproj(0))
            drain(gen_vchain())
            for j in range(4):
                drain(gen_scores(j))
                if j == 0:
                    drain(gen_s1_late(0))
                gen_rb_evac(j)
                if j < 3:
                    drain(gen_qkproj(j + 1))
                drain(rest(j))

    nc.compile()
    return nc


_graph_cache = None


def _get_graph():
    global _graph_cache
    if _graph_cache is None:
        _graph_cache = build_graph()
    return _graph_cache


# ---------------------------------------------------------------- host ----
def _prep_core_inputs(c, x, Wq, bq, Wk, bk, Wv, bv, Wo, coeffs):
    bf = ml_dtypes.bfloat16
    b, hh = c // 2, c % 2
    heads = [4 * hh + j for j in range(4)]

    xTb = np.ascontiguousarray(x[b].T.astype(np.float32)).reshape(2, 128, N)

    def rep4_rearr(Wcols):  # [256, 32] -> tiled x4 -> [128, 2, 128]
        wrep = np.tile(Wcols, (1, 4))                      # [256, 128]
        return np.ascontiguousarray(
            wrep.reshape(2, 128, 128).transpose(1, 0, 2))  # [128, 2, 128]

    WQc = np.stack([rep4_rearr(Wq[:, 32 * h:32 * h + 32] * SCALE)
                    for h in heads]).astype(bf)            # [4, 128, 2, 128]
    WKc = np.stack([rep4_rearr(Wk[:, 32 * h:32 * h + 32])
                    for h in heads]).astype(bf)

    wv_cols = np.concatenate([Wv[:, 32 * h:32 * h + 32] for h in heads], 1)
    WVc = np.ascontiguousarray(
        wv_cols.reshape(2, 128, 128).transpose(1, 0, 2)).astype(bf)

    WOKc = np.zeros((4, 128, 4, 256), np.float32)
    for j, h in enumerate(heads):
        rows = Wo[32 * h:32 * h + 32, :]                   # [32, 256]
        for k in range(4):
            WOKc[j, :, k, :] = np.tile(coeffs[h, k] * rows, (4, 1))

    return {"xT": xTb.astype(bf), "WQ": WQc, "WK": WKc,
            "WV": WVc, "WOK": WOKc.astype(bf)}


def kernel(**inputs):
    x = np.asarray(inputs["x"], np.float32)
    Wq = np.asarray(inputs["Wq"], np.float32)
    bq = np.asarray(inputs["bq"], np.float32)
    Wk = np.asarray(inputs["Wk"], np.float32)
    bk = np.asarray(inputs["bk"], np.float32)
    Wv = np.asarray(inputs["Wv"], np.float32)
    bv = np.asarray(inputs["bv"], np.float32)
    Wo = np.asarray(inputs["Wo"], np.float32)
    bo = np.asarray(inputs["bo"], np.float32)
    coeffs = np.asarray(inputs["coeffs"], np.float32)

    nc = _get_graph()
    in_maps = [_prep_core_inputs(c, x, Wq, bq, Wk, bk, Wv, bv, Wo, coeffs)
               for c in range(8)]
    res = run_bass_kernel_spmd(nc, in_maps, core_ids=list(range(8))).results

    out = np.zeros((B, N, D), np.float32)
    for c in range(8):
        o = np.asarray(res[c]["out"], np.float32)     # [4, 2, 128, N]
        out[c // 2] += o.sum(axis=0).reshape(256, N).T
    out += bo[None, None, :]
    return out
